# revision 11
# baseline (speedup 1.0000x reference)
"""LIF layer (leaky integrate-and-fire scan over time) on 8 Trainium2 cores.

Recurrence per (b, f) row over t = 0..L-1:
    v_pre[t] = alpha[f] * v[t-1] + (1 - alpha[f]) * I[b, f, t]
    z[t]     = BETA * (v_pre[t] - THR)
    s[t]     = (v_pre[t] >= THR)
    v[t]     = v_pre[t] * (v_pre[t] < THR)          # reset on spike

Outputs: (v_pre, z, s) each [B, F, L] float32.

Sharding: pure data parallel over a (B x F) grid -- B split SB ways, F split
SF ways (SB*SF = 8 cores). Per core: [BL, FL, L] with partition dim = f so
alpha is a per-partition [FL, 1] scalar operand of the fused
scalar_tensor_tensor DVE ops that implement the sequential scan (2 per step).
J = (1-alpha)*I precompute runs on ScalarE; z and s are bulk ops on GpSimd;
DMA on SyncE. Everything except the 2-op serial DVE chain is overlapped.
"""

import sys

sys.path.insert(0, "/opt/trn_rl_repo")

import numpy as np

DT = 1.0
BETA = 15.0
THR = 0.25

B, F, L = 64, 256, 2048
SB, SF = 4, 2  # B-split x F-split = 8 cores
BL, FL = B // SB, F // SF  # 16, 128
TC = 256  # time-chunk length
N_CORES = SB * SF

_BUILD_CACHE: dict = {}
LAST_RESULTS = None  # BassKernelResults of the most recent kernel() call


def _build(bl: int, fl: int, ll: int, tc: int):
    """Build the per-core Bass program (same NEFF for all cores)."""
    import concourse.bacc as bacc
    import concourse.mybir as mybir
    from concourse import tile

    f32 = mybir.dt.float32
    Alu = mybir.AluOpType
    Act = mybir.ActivationFunctionType

    nchunk = ll // tc
    assert ll % tc == 0

    nc = bacc.Bacc(None, target_bir_lowering=False)
    i_d = nc.dram_tensor("i_loc", [fl, bl, ll], f32, kind="ExternalInput")
    al_d = nc.dram_tensor("alpha", [fl, 1], f32, kind="ExternalInput")
    om_d = nc.dram_tensor("omalpha", [fl, 1], f32, kind="ExternalInput")
    v_d = nc.dram_tensor("v_out", [fl, bl, ll], f32, kind="ExternalOutput")
    z_d = nc.dram_tensor("z_out", [fl, bl, ll], f32, kind="ExternalOutput")
    s_d = nc.dram_tensor("s_out", [fl, bl, ll], f32, kind="ExternalOutput")

    with tile.TileContext(nc) as tc_:
        with (
            tc_.tile_pool(name="const", bufs=1) as constp,
            tc_.tile_pool(name="io", bufs=2) as iop,
        ):
            al_t = constp.tile([fl, 1], f32, tag="al")
            om_t = constp.tile([fl, 1], f32, tag="om")
            nc.sync.dma_start(al_t[:], al_d[:])
            nc.sync.dma_start(om_t[:], om_d[:])

            vst = constp.tile([fl, bl], f32, tag="vst")
            nc.gpsimd.memset(vst[:], 0.0)

            for k in range(nchunk):
                tsl = slice(k * tc, (k + 1) * tc)

                it = iop.tile([fl, bl, tc], f32, tag="i")
                nc.sync.dma_start(it[:], i_d[:, :, tsl])

                # J = (1 - alpha) * I  (single-rounded FMA on ScalarE; same
                # result as the reference's f32 multiply)
                jt = iop.tile([fl, bl, tc], f32, tag="j")
                nc.scalar.activation(jt[:], it[:], Act.Copy, bias=0.0, scale=om_t[:, 0:1])

                vp = iop.tile([fl, bl, tc], f32, tag="vp")
                for t in range(tc):
                    # v_pre = (v * alpha) + J_t
                    nc.vector.scalar_tensor_tensor(
                        vp[:, :, t], vst[:], al_t[:, 0:1], jt[:, :, t],
                        op0=Alu.mult, op1=Alu.add,
                    )
                    # v = (v_pre < thr) * v_pre
                    nc.vector.scalar_tensor_tensor(
                        vst[:], vp[:, :, t], THR, vp[:, :, t],
                        op0=Alu.is_lt, op1=Alu.mult,
                    )

                # z = (v_pre - thr) * BETA   (reference rounding order)
                zt = iop.tile([fl, bl, tc], f32, tag="z")
                nc.gpsimd.tensor_scalar(zt[:], vp[:], THR, BETA, Alu.subtract, Alu.mult)
                # s = (v_pre >= thr)
                st = iop.tile([fl, bl, tc], f32, tag="s")
                nc.gpsimd.tensor_scalar(st[:], vp[:], THR, None, Alu.is_ge)

                nc.sync.dma_start(v_d[:, :, tsl], vp[:])
                nc.sync.dma_start(z_d[:, :, tsl], zt[:])
                nc.sync.dma_start(s_d[:, :, tsl], st[:])

    nc.compile()
    return nc


def _get_nc():
    key = (BL, FL, L, TC)
    if key not in _BUILD_CACHE:
        _BUILD_CACHE[key] = _build(*key)
    return _BUILD_CACHE[key]


def _build_v2(bl: int, fl: int, tseg: int, w: int, tc: int):
    """Time-sharded build: 8 cores = 2 f-halves x 4 time segments.

    Each core scans w warmup steps (converging the decaying state from
    v=0; seg 0 gets zero-padded input so the NEFF is uniform) and then
    tseg output steps. Serial chain: 2 fused STT DVE ops per step at
    free-dim = bl.

    All DRAM I/O is slab-major — [fl, n_slabs, bl, tc] — so every DMA
    moves one whole [fl, bl*tc] tile as 128 contiguous per-partition
    slabs (16KB descriptors), letting short chunks stream without the
    sub-512B descriptor penalty. The host packs/unpacks the layout.
    """
    import concourse.bacc as bacc
    import concourse.mybir as mybir
    from concourse import tile

    f32 = mybir.dt.float32
    Alu = mybir.AluOpType
    Act = mybir.ActivationFunctionType

    tt = w + tseg
    assert tt % tc == 0 and w % tc == 0
    nw, ns = w // tc, tseg // tc

    nc = bacc.Bacc(None, target_bir_lowering=False)
    i_d = nc.dram_tensor("i_loc", [fl, nw + ns, bl, tc], f32, kind="ExternalInput")
    al_d = nc.dram_tensor("alpha", [fl, 1], f32, kind="ExternalInput")
    om_d = nc.dram_tensor("omalpha", [fl, 1], f32, kind="ExternalInput")
    v_d = nc.dram_tensor("v_out", [fl, ns, bl, tc], f32, kind="ExternalOutput")
    z_d = nc.dram_tensor("z_out", [fl, ns, bl, tc], f32, kind="ExternalOutput")
    s_d = nc.dram_tensor("s_out", [fl, ns, bl, tc], f32, kind="ExternalOutput")

    with tile.TileContext(nc) as tc_:
        with (
            tc_.tile_pool(name="const", bufs=1) as constp,
            tc_.tile_pool(name="io", bufs=3) as iop,
            tc_.tile_pool(name="zs", bufs=2) as zsp,
        ):
            al_t = constp.tile([fl, 1], f32, tag="al")
            om_t = constp.tile([fl, 1], f32, tag="om")
            nc.sync.dma_start(al_t[:], al_d[:])
            nc.sync.dma_start(om_t[:], om_d[:])

            vst = constp.tile([fl, bl], f32, tag="vst")
            nc.gpsimd.memset(vst[:], 0.0)
            vp_w = constp.tile([fl, bl], f32, tag="vpw")  # warmup v_pre slot

            for k in range(nw + ns):
                is_out = k >= nw
                it = iop.tile([fl, bl, tc], f32, tag="i")
                nc.sync.dma_start(it[:], i_d[:, k])
                # J = (1 - alpha) * I, in place over the input tile
                nc.scalar.activation(it[:], it[:], Act.Copy, bias=0.0, scale=om_t[:, 0:1])

                if not is_out:  # warmup chunk: no outputs
                    for t in range(tc):
                        nc.vector.scalar_tensor_tensor(
                            vp_w[:], vst[:], al_t[:, 0:1], it[:, :, t],
                            op0=Alu.mult, op1=Alu.add,
                        )
                        nc.vector.scalar_tensor_tensor(
                            vst[:], vp_w[:], THR, vp_w[:],
                            op0=Alu.is_lt, op1=Alu.mult,
                        )
                    continue

                last = k == nw + ns - 1
                o = k - nw
                vp = iop.tile([fl, bl, tc], f32, tag="vp")
                for t in range(tc):
                    nc.vector.scalar_tensor_tensor(
                        vp[:, :, t], vst[:], al_t[:, 0:1], it[:, :, t],
                        op0=Alu.mult, op1=Alu.add,
                    )
                    nc.vector.scalar_tensor_tensor(
                        vst[:], vp[:, :, t], THR, vp[:, :, t],
                        op0=Alu.is_lt, op1=Alu.mult,
                    )

                # z = (vp - thr) * beta, s = (vp >= thr): bulk on GpSimd
                # mid-stream (hidden behind the DVE chain); on DVE for the
                # final chunk so the tail isn't gated on slow GpSimd passes.
                eng = nc.vector if last else nc.gpsimd
                zt = zsp.tile([fl, bl, tc], f32, tag="z")
                eng.tensor_scalar(zt[:], vp[:], THR, BETA, Alu.subtract, Alu.mult)
                st = zsp.tile([fl, bl, tc], f32, tag="s")
                eng.tensor_scalar(st[:], vp[:], THR, None, Alu.is_ge)

                # Outputs ride the ACT HWDGE ring so they never queue ahead
                # of the next input chunk on the SP ring (FIFO per ring).
                nc.scalar.dma_start(v_d[:, o], vp[:])
                nc.scalar.dma_start(z_d[:, o], zt[:])
                nc.scalar.dma_start(s_d[:, o], st[:])

    nc.compile()
    return nc


def _pick_warmup(alpha: np.ndarray) -> int:
    """Steps for the state to converge below fp32 resolution from v=0,
    with ~2x margin for spike-flip self-healing. Multiple of 128."""
    amax = float(alpha.max())
    amax = min(max(amax, 1e-6), 0.999999)
    wraw = 2.2 * np.log(4e-10) / np.log(amax)
    w = int(np.ceil(max(wraw, 1.0) / 128.0)) * 128
    return max(w, 128)


def _alpha_host(raw_tau: np.ndarray) -> tuple[np.ndarray, np.ndarray]:
    """alpha = exp(-DT / (softplus(raw_tau) + 1e-4)) with the same jax ops /
    device as the reference, so spike threshold comparisons match bitwise."""
    import jax
    import jax.numpy as jnp

    with jax.default_device(jax.devices("cpu")[0]):
        tau = jax.nn.softplus(jnp.asarray(np.asarray(raw_tau))) + 1e-4
        alpha = np.asarray(jnp.exp(-DT / tau), dtype=np.float32)
    one_minus = (np.float32(1.0) - alpha).astype(np.float32)
    return alpha, one_minus


USE_V2 = True
_CURRENT_NC = None


def _get_current_nc():
    return _CURRENT_NC


def _run_v1(I, alpha, one_minus, _trace):
    global LAST_RESULTS, _CURRENT_NC
    from concourse.bass_utils import run_bass_kernel_spmd

    nc = _get_nc()
    _CURRENT_NC = nc

    in_maps = []
    for c in range(N_CORES):
        fg, bg = c % SF, c // SF
        fsl = slice(fg * FL, (fg + 1) * FL)
        bsl = slice(bg * BL, (bg + 1) * BL)
        i_loc = np.ascontiguousarray(I[bsl, fsl, :].transpose(1, 0, 2))  # [FL, BL, L]
        in_maps.append(
            {
                "i_loc": i_loc,
                "alpha": np.ascontiguousarray(alpha[fsl].reshape(FL, 1)),
                "omalpha": np.ascontiguousarray(one_minus[fsl].reshape(FL, 1)),
            }
        )

    res = run_bass_kernel_spmd(nc, in_maps, core_ids=list(range(N_CORES)), trace=_trace)
    LAST_RESULTS = res

    v = np.empty((B, F, L), np.float32)
    z = np.empty((B, F, L), np.float32)
    s = np.empty((B, F, L), np.float32)
    for c in range(N_CORES):
        fg, bg = c % SF, c // SF
        fsl = slice(fg * FL, (fg + 1) * FL)
        bsl = slice(bg * BL, (bg + 1) * BL)
        r = res.results[c]
        v[bsl, fsl, :] = r["v_out"].transpose(1, 0, 2)
        z[bsl, fsl, :] = r["z_out"].transpose(1, 0, 2)
        s[bsl, fsl, :] = r["s_out"].transpose(1, 0, 2)
    return v, z, s


def _run_v2(I, alpha, one_minus, w, _trace):
    global LAST_RESULTS, _CURRENT_NC
    from concourse.bass_utils import run_bass_kernel_spmd

    nseg = 4
    tseg = L // nseg  # 512
    bl2, fl2, tc = B, 128, 64  # all of B, half of F per core

    key = ("v2", bl2, fl2, tseg, w, tc)
    if key not in _BUILD_CACHE:
        _BUILD_CACHE[key] = _build_v2(bl2, fl2, tseg, w, tc)
    nc = _BUILD_CACHE[key]
    _CURRENT_NC = nc

    nck = (w + tseg) // tc
    in_maps = []
    for c in range(N_CORES):
        fg, seg = c % 2, c // 2
        fsl = slice(fg * fl2, (fg + 1) * fl2)
        t0 = seg * tseg
        i_pad = np.zeros((fl2, bl2, w + tseg), np.float32)
        lo = max(0, t0 - w)
        i_pad[:, :, w - (t0 - lo):] = I[:, fsl, lo : t0 + tseg].transpose(1, 0, 2)
        i_sm = i_pad.reshape(fl2, bl2, nck, tc).transpose(0, 2, 1, 3)
        in_maps.append(
            {
                "i_loc": np.ascontiguousarray(i_sm),
                "alpha": np.ascontiguousarray(alpha[fsl].reshape(fl2, 1)),
                "omalpha": np.ascontiguousarray(one_minus[fsl].reshape(fl2, 1)),
            }
        )

    res = run_bass_kernel_spmd(nc, in_maps, core_ids=list(range(N_CORES)), trace=_trace)
    LAST_RESULTS = res

    v = np.empty((B, F, L), np.float32)
    z = np.empty((B, F, L), np.float32)
    s = np.empty((B, F, L), np.float32)
    for c in range(N_CORES):
        fg, seg = c % 2, c // 2
        fsl = slice(fg * fl2, (fg + 1) * fl2)
        t0 = seg * tseg
        r = res.results[c]
        for name, dst in (("v_out", v), ("z_out", z), ("s_out", s)):
            a = r[name].transpose(2, 0, 1, 3).reshape(bl2, fl2, tseg)
            dst[:, fsl, t0 : t0 + tseg] = a
    return v, z, s


def kernel(I: np.ndarray, raw_tau: np.ndarray, _trace: bool = False):
    I = np.asarray(I, dtype=np.float32)
    raw_tau = np.asarray(raw_tau, dtype=np.float32)
    assert I.shape == (B, F, L), I.shape

    alpha, one_minus = _alpha_host(raw_tau)
    w = _pick_warmup(alpha)
    if USE_V2 and w <= 512:
        return _run_v2(I, alpha, one_minus, w, _trace)
    return _run_v1(I, alpha, one_minus, _trace)
